# revision 16
# baseline (speedup 1.0000x reference)
# Self-contained Trainium2 Bass kernel for the LN->QKV->sparse-rel-pos-attention->proj block.
#
# Reference computation (B=128, N=256, DIM=512, H=12, KD=32, D=128):
#   xn   = LayerNorm(x) * gamma + beta
#   qkv  = xn @ Wqkv + bqkv ; split q,k,v per head
#   attn = softmax(q k^T / sqrt(KD) + biases[:, bias_idxs])
#   out  = (attn @ v) @ Wproj + bproj
#
# Strategy: pure data-parallel over batch across 8 NeuronCores (16 elems/core).
# Host folds: gamma/beta into Wqkv, 1/sqrt(KD) into Wq, v-bias into bproj,
# and expands exp(biases[:, bias_idxs]) so softmax(S+B) = expS*expB row-normalized.
# Device layouts avoid all transposes except the initial z -> z^T:
#   qk^T [feat, tok] and v [tok, feat] both come from matmuls against z^T;
#   S^T = k q^T has tokens-m on partitions so exp/Z/AV consume it directly;
#   AV gives O^T [head-dim, tok] which is exactly proj's stationary layout.
# All matmul operands are fp16 (PSUM accumulation stays fp32) so every matmul
# runs at the 2.4 GHz / 1-cycle-per-row stream rate. Scheduling keeps the PE
# queue dense: LN stats for pair p+1 are spread one-tile-at-a-time across
# pair p's attention groups, the Sqrt lands in the proj window (its two ACT
# table reloads hide behind the matmul-only transpose/QKV phase), pair p+1's
# transposes are emitted inside pair p's proj phase, and the S matmuls for
# group g+1 are issued before Z/AV of group g to cover the exp->mult latency.
# Softmax normalizer: Z^T = P^T-colsum via ones-matmul, reciprocal on DVE,
# DMA partition-broadcast, one fused DVE multiply on the AV output.

import numpy as np

B, N, DIM = 128, 256, 512
H, KD = 12, 32
D = 128
DH = D * H
RES = 16
EPS = 1e-5
NCORES = 8
BPC = B // NCORES

_CACHE = {}

# heads processed in strip-pure pairs: strips (h % 3) equal within each pair
HEAD_ORDER = [0, 3, 6, 9, 1, 4, 7, 10, 2, 5, 8, 11]


def _build(bpc, use_bqk, use_bp):
    from contextlib import ExitStack

    import concourse.bacc as bacc
    import concourse.tile as tile
    from concourse import mybir
    from concourse.masks import make_identity

    f32 = mybir.dt.float32
    f16 = mybir.dt.float16
    Alu = mybir.AluOpType
    Act = mybir.ActivationFunctionType

    nc = bacc.Bacc("TRN2", target_bir_lowering=False, debug=False,
                   num_devices=NCORES)

    x_d = nc.dram_tensor("x", [bpc, N, DIM], f32, kind="ExternalInput").ap()
    wqk_d = nc.dram_tensor("wqk", [DIM, 8 * 128], f16, kind="ExternalInput").ap()
    wv_d = nc.dram_tensor("wv", [DIM, DH], f16, kind="ExternalInput").ap()
    wp_d = nc.dram_tensor("wp", [DH, DIM], f16, kind="ExternalInput").ap()
    expb_d = nc.dram_tensor("expb", [128, 2, H, N], f16, kind="ExternalInput").ap()
    ones_d = nc.dram_tensor("ones", [128, 1], f16, kind="ExternalInput").ap()
    if use_bqk:
        bqk_d = nc.dram_tensor("bqk", [128, 8], f32, kind="ExternalInput").ap()
    if use_bp:
        bp_d = nc.dram_tensor("bp", [DIM], f32, kind="ExternalInput").ap()
    y_d = nc.dram_tensor("y", [bpc, N, DIM], f32, kind="ExternalOutput").ap()

    with tile.TileContext(nc) as tc, ExitStack() as ctx:
        consts = ctx.enter_context(tc.tile_pool(name="consts", bufs=1))
        sb_x = ctx.enter_context(tc.tile_pool(name="sb_x", bufs=2))
        sb_xn = ctx.enter_context(tc.tile_pool(name="sb_xn", bufs=8))
        sb_zT = ctx.enter_context(tc.tile_pool(name="sb_zT", bufs=2))
        sb_qkT = ctx.enter_context(tc.tile_pool(name="sb_qkT", bufs=1))
        sb_v = ctx.enter_context(tc.tile_pool(name="sb_v", bufs=2))
        sb_pt = ctx.enter_context(tc.tile_pool(name="sb_pt", bufs=4))
        sb_zb = ctx.enter_context(tc.tile_pool(name="sb_zb", bufs=2))
        sb_ot = ctx.enter_context(tc.tile_pool(name="sb_ot", bufs=2))
        sb_y = ctx.enter_context(tc.tile_pool(name="sb_y", bufs=2))
        sb_small = ctx.enter_context(tc.tile_pool(name="sb_small", bufs=3))
        ps_work = ctx.enter_context(tc.tile_pool(name="ps_work", bufs=2, space="PSUM"))
        ps_s = ctx.enter_context(tc.tile_pool(name="ps_s", bufs=3, space="PSUM"))
        ps_oty = ctx.enter_context(tc.tile_pool(name="ps_oty", bufs=3, space="PSUM"))
        dram = ctx.enter_context(tc.tile_pool(name="dram", bufs=2, space="DRAM"))

        # ---- constants.  The first pair's x tiles + LN chain are emitted
        # before the big weight DMAs so the transposes can start ~early;
        # weight DMA order follows first-use order (wqk, wv, expb, wp).
        ident = consts.tile([128, 128], f16)
        make_identity(nc, ident)
        ones_col = consts.tile([128, 1], f16)
        nc.sync.dma_start(out=ones_col, in_=ones_d)
        eps_t = consts.tile([128, 1], f32)
        nc.vector.memset(eps_t, EPS)
        if use_bqk:
            bqk_sb = consts.tile([128, 8], f32)
            nc.sync.dma_start(out=bqk_sb, in_=bqk_d)
        if use_bp:
            bp_sb = consts.tile([128, 1, DIM], f32)
            nc.sync.dma_start(out=bp_sb, in_=bp_d.partition_broadcast(128))

        assert bpc % 2 == 0
        npairs = bpc // 2

        def emit_stats_part(ep, i, st):
            # load one x tile and compute its bn stats (spread across the
            # attention groups so the DVE queue never blocks on a burst)
            if i == 0:
                st["mv"] = sb_small.tile([128, 2, 2, 2], f32, tag="mv",
                                         bufs=2, name="mv")
                st["x_ts"] = []
            el, tci = i // 2, i % 2
            x_t = sb_x.tile([128, DIM], f32, tag="x", bufs=8)
            nc.sync.dma_start(
                out=x_t,
                in_=x_d[2 * ep + el, tci * 128:(tci + 1) * 128, :])
            stats = sb_small.tile([128, 6], f32, tag="stats")
            nc.vector.bn_stats(stats, x_t)
            nc.vector.bn_aggr(st["mv"][:, el, tci, :], stats)
            st["x_ts"].append(x_t)

        def emit_norm(x_ts, mv):
            # single Sqrt per pair (table reloads between Sqrt and Exp are
            # ~1.3us each), then normalize into fp16 tiles
            sig = sb_small.tile([128, 2, 2], f32, tag="sig", bufs=2)
            nc.scalar.activation(sig, mv[:, :, :, 1], Act.Sqrt, bias=eps_t,
                                 scale=1.0)
            rsig = sb_small.tile([128, 2, 2], f32, tag="rsig", bufs=2)
            nc.vector.reciprocal(rsig, sig)
            xn_ts = []
            for el in range(2):
                for tci in range(2):
                    x_t = x_ts[2 * el + tci]
                    xn_t = sb_xn.tile([128, DIM], f16, tag="xn")
                    nc.gpsimd.tensor_scalar(out=xn_t, in0=x_t,
                                            scalar1=mv[:, el, tci, 0:1],
                                            scalar2=rsig[:, el, tci:tci + 1],
                                            op0=Alu.subtract, op1=Alu.mult)
                    xn_ts.append(xn_t)
            return xn_ts

        def alloc_zT():
            return sb_zT.tile([128, 4, 2 * N], f16, tag="zT", name="zT_sb")

        def emit_transposes(zT_sb, xn_ts, el):
            # transposes time-share the S-matmul PSUM ring (fp16 view of the
            # f32 bank tile) — they run in the proj window when S is idle
            s_t = ps_s.tile([128, 512], f32, tag="s", name="s_ps")
            zT_ps = s_t.bitcast(f16).rearrange("p (a kc t) -> p a kc t",
                                               a=2, kc=4)
            for tci in range(2):
                xn_t = xn_ts[2 * el + tci]
                for kc in range(4):
                    nc.tensor.transpose(zT_ps[:, tci, kc, :],
                                        xn_t[:, kc * 128:(kc + 1) * 128],
                                        ident)
            for tci in range(2):
                off = el * N + tci * 128
                nc.scalar.activation(zT_sb[:, :, off:off + 128],
                                     zT_ps[:, tci, :, :], Act.Copy)

        st0 = {}
        for i in range(4):
            emit_stats_part(0, i, st0)
        xn_cur = emit_norm(st0["x_ts"], st0["mv"])

        # weight DMAs queue after the first pair's x so the LN/transpose
        # prologue is not stuck behind 5.5MB of weights
        wqk_sb = consts.tile([128, 4, 8 * 128], f16)
        nc.sync.dma_start(out=wqk_sb, in_=wqk_d.rearrange("(kc p) f -> p kc f", p=128))
        wv_sb = consts.tile([128, 4, DH], f16)
        nc.sync.dma_start(out=wv_sb, in_=wv_d.rearrange("(kc p) f -> p kc f", p=128))
        expb_sb = consts.tile([128, 2, H, N], f16)
        nc.sync.dma_start(out=expb_sb, in_=expb_d)
        wp_sb = consts.tile([128, H, DIM], f16)
        nc.sync.dma_start(out=wp_sb, in_=wp_d.rearrange("(h p) f -> p h f", p=128))

        zT_cur = alloc_zT()
        emit_transposes(zT_cur, xn_cur, 0)
        emit_transposes(zT_cur, xn_cur, 1)

        for ep in range(npairs):
            st_next = {}
            zT_sb = zT_cur

            # ---- qk^T = W'' ^T z^T   [feat, tok-pair].  Head h's q lives in
            # chunk h//3, its k in chunk 4 + h//3, both at 32-row strip h%3.
            # The S matmuls contract K=32 at partition base 32*(h%3); heads
            # are processed in strip-pure pairs because interleaving different
            # PE tile_positions on one PSUM bank hangs the device
            # (sem-separated bank reuse across strips is fine).
            qkT_sb = sb_qkT.tile([128, 8, 2 * N], f16, tag="qkT", bufs=1)
            for fc in range(8):
                qk_ps = ps_work.tile([128, 512], f32, tag="work")
                for kc in range(4):
                    nc.tensor.matmul(qk_ps,
                                     lhsT=wqk_sb[:, kc, fc * 128:(fc + 1) * 128],
                                     rhs=zT_sb[:, kc, :],
                                     start=(kc == 0), stop=(kc == 3))
                # GpSimd has no PSUM access path; split PSUM copies ACT/DVE
                if fc % 2 == 0:
                    nc.scalar.activation(qkT_sb[:, fc, :], qk_ps, Act.Copy)
                else:
                    nc.vector.tensor_copy(out=qkT_sb[:, fc, :], in_=qk_ps)
                if use_bqk:
                    nc.vector.tensor_scalar_add(
                        out=qkT_sb[:, fc, :], in0=qkT_sb[:, fc, :],
                        scalar1=bqk_sb[:, fc:fc + 1])

            xn_next = None
            zT_next = None

            for el in range(2):
                e = 2 * ep + el
                etok = el * N
                # ---- v = z Wv   [tok 256, feat 1536] ----
                v_sb = sb_v.tile([128, 2, DH], f16, tag="v")
                for mc in range(2):
                    for ns in range(3):
                        v_ps = ps_work.tile([128, 512], f32, tag="work")
                        for kc in range(4):
                            nc.tensor.matmul(
                                v_ps,
                                lhsT=zT_sb[:, kc,
                                           etok + mc * 128:etok + (mc + 1) * 128],
                                rhs=wv_sb[:, kc, ns * 512:(ns + 1) * 512],
                                start=(kc == 0), stop=(kc == 3))
                        if (mc * 3 + ns) % 2 == 0:
                            nc.scalar.activation(
                                v_sb[:, mc, ns * 512:(ns + 1) * 512], v_ps,
                                Act.Copy)
                        else:
                            nc.vector.tensor_copy(
                                out=v_sb[:, mc, ns * 512:(ns + 1) * 512],
                                in_=v_ps)

                # ---- attention, strip-pure head pairs ----
                # slot 2g+hl in pt/ot/expb corresponds to HEAD_ORDER[2g+hl]
                ot_sb = sb_ot.tile([128, H, N], f16, tag="ot")
                pt_tiles = [None] * 6

                def emit_S(g):
                    # S matmuls + exp + expB multiply for group g
                    pt_sb = sb_pt.tile([128, 2, 2, N], f16, tag="pt",
                                       name="pt_sb")
                    pt_tiles[g] = pt_sb
                    for mc in range(2):
                        s_ps = ps_s.tile([128, 512], f32, tag="s", name="s_ps")
                        for hl in range(2):
                            h = HEAD_ORDER[2 * g + hl]
                            qc = h // 3
                            base = (h % 3) * KD
                            nc.tensor.matmul(
                                s_ps[:, hl * N:(hl + 1) * N],
                                lhsT=qkT_sb[base:base + KD, 4 + qc,
                                            etok + mc * 128:etok + (mc + 1) * 128],
                                rhs=qkT_sb[base:base + KD, qc, etok:etok + N],
                                start=True, stop=True)
                        nc.scalar.activation(pt_sb[:, mc],
                                             s_ps.rearrange("p (a n) -> p a n",
                                                            a=2),
                                             Act.Exp)
                        # alternate the expB multiply between GpSimd and DVE
                        eng = nc.gpsimd if (g + mc) % 2 == 0 else nc.vector
                        eng.tensor_tensor(out=pt_sb[:, mc], in0=pt_sb[:, mc],
                                          in1=expb_sb[:, mc, 2 * g:2 * g + 2, :],
                                          op=Alu.mult)

                emit_S(0)
                for g in range(6):
                    # issue the next group's S/exp/mult first so the PE has
                    # work while group g's softmax chain drains
                    if g + 1 < 6:
                        emit_S(g + 1)
                    pt_sb = pt_tiles[g]
                    # Z[hl, n] = sum_m P^T[m, n]; one matmul per mc chunk
                    zrow_ps = ps_oty.tile([128, 512], f32, tag="oty",
                                          name="zrow_ps")
                    for mc in range(2):
                        nc.tensor.matmul(zrow_ps[0:1, :],
                                         lhsT=ones_col,
                                         rhs=pt_sb[:, mc, :, :].rearrange(
                                             "p a n -> p (a n)"),
                                         start=(mc == 0), stop=(mc == 1))
                    zrecip_sb = sb_small.tile([1, 512], f32, tag="zrecip")
                    nc.vector.reciprocal_approx_fast(out=zrecip_sb,
                                                     in_=zrow_ps[0:1, :])
                    # partition-broadcast via DRAM roundtrip (step-0 partition
                    # APs are only legal on DRAM sources)
                    zscr = dram.tile([1, 512], f32, tag="zscr")
                    nc.sync.dma_start(out=zscr, in_=zrecip_sb)
                    zb_sb = sb_zb.tile([128, 2, N], f32, tag="zb")
                    nc.sync.dma_start(out=zb_sb,
                                      in_=zscr[0, :].partition_broadcast(128))
                    # O^T = v^T P^T  (normalized by zb afterwards)
                    ot_ps = ps_oty.tile([128, 512], f32, tag="oty",
                                        name="ot_ps")
                    for hl in range(2):
                        h = HEAD_ORDER[2 * g + hl]
                        for mc in range(2):
                            nc.tensor.matmul(
                                ot_ps[:, hl * N:(hl + 1) * N],
                                lhsT=v_sb[:, mc, h * 128:(h + 1) * 128],
                                rhs=pt_sb[:, mc, hl, :],
                                start=(mc == 0), stop=(mc == 1))
                    nc.vector.tensor_tensor(
                        out=ot_sb[:, 2 * g:2 * g + 2, :],
                        in0=ot_ps.rearrange("p (a n) -> p a n", a=2),
                        in1=zb_sb, op=Alu.mult)
                    # next pair's LN stats, one x-tile at a time so the DVE
                    # queue never stalls the attention chain
                    if ep + 1 < npairs:
                        if el == 0 and g % 2 == 1:
                            emit_stats_part(ep + 1, g // 2, st_next)
                        elif el == 1 and g == 1:
                            emit_stats_part(ep + 1, 3, st_next)

                # the Sqrt (and its two ACT table reloads) lands in the
                # proj/transpose window where the ACT engine runs no Exp
                if el == 1 and ep + 1 < npairs:
                    xn_next = emit_norm(st_next["x_ts"], st_next["mv"])

                # ---- proj: y = O Wp ----  (next pair's transposes are
                # emitted between the two proj chains to fill PE gaps)
                for nci in range(2):
                    y_ps = ps_oty.tile([128, 512], f32, tag="oty", name="y_ps")
                    for slot in range(H):
                        nc.tensor.matmul(y_ps,
                                         lhsT=ot_sb[:, slot, nci * 128:(nci + 1) * 128],
                                         rhs=wp_sb[:, HEAD_ORDER[slot], :],
                                         start=(slot == 0), stop=(slot == H - 1))
                    yb_sb = sb_y.tile([128, DIM], f32, tag="yb")
                    if use_bp:
                        nc.vector.tensor_tensor(out=yb_sb, in0=y_ps,
                                                in1=bp_sb[:, 0, :], op=Alu.add)
                    else:
                        nc.scalar.activation(yb_sb, y_ps, Act.Copy)
                    nc.sync.dma_start(out=y_d[e, nci * 128:(nci + 1) * 128, :],
                                      in_=yb_sb)
                    if el == 1 and ep + 1 < npairs:
                        if nci == 0:
                            zT_next = alloc_zT()
                        emit_transposes(zT_next, xn_next, nci)

            xn_cur = xn_next
            zT_cur = zT_next

    nc.compile()
    return nc


def _prepare(x, gamma, beta, Wqkv, bqkv, Wproj, bproj, biases, bias_idxs):
    x = np.ascontiguousarray(np.asarray(x, dtype=np.float32))
    gamma = np.asarray(gamma, dtype=np.float32)
    beta = np.asarray(beta, dtype=np.float32)
    Wqkv = np.asarray(Wqkv, dtype=np.float32)
    bqkv = np.asarray(bqkv, dtype=np.float32)
    Wproj = np.asarray(Wproj, dtype=np.float32)
    bproj = np.asarray(bproj, dtype=np.float32)
    biases = np.asarray(biases, dtype=np.float32)
    bias_idxs = np.asarray(bias_idxs)

    s = np.float32(KD ** -0.5)
    Wg = Wqkv * gamma[:, None]
    bfull = beta @ Wqkv + bqkv
    Wr = Wg.reshape(DIM, H, 64 + D)
    br = bfull.reshape(H, 64 + D)
    # feature layout (see kernel comment): head h -> strip h%3; q in chunk
    # h//3, k in chunk 4 + h//3.
    wqk = np.zeros((DIM, 8, 128), dtype=np.float32)
    bqk = np.zeros((8, 128), dtype=np.float32)
    for h in range(H):
        qc, base = h // 3, (h % 3) * KD
        wqk[:, qc, base:base + KD] = Wr[:, h, 0:KD] * s
        wqk[:, 4 + qc, base:base + KD] = Wr[:, h, KD:2 * KD]
        bqk[qc, base:base + KD] = br[h, 0:KD] * s
        bqk[4 + qc, base:base + KD] = br[h, KD:2 * KD]
    wqk = np.ascontiguousarray(wqk.reshape(DIM, 8 * 128)).astype(np.float16)
    wv = np.ascontiguousarray(
        Wr[:, :, 2 * KD:].reshape(DIM, DH)).astype(np.float16)
    bv = br[:, 2 * KD:].reshape(DH)
    bp = bproj + bv @ Wproj
    expb = np.exp(biases[:, bias_idxs])  # [H, N, N]
    # head dim reordered to the kernel's strip-pure processing order
    expb_t = np.ascontiguousarray(
        expb[HEAD_ORDER].reshape(H, 2, 128, N).transpose(2, 1, 0, 3)
    ).astype(np.float16)

    use_bqk = bool(np.abs(bqk).max() > 0)
    use_bp = bool(np.abs(bp).max() > 0)
    bqk_t = np.ascontiguousarray(bqk.T)  # [128, 8]

    common = {"wqk": wqk, "wv": wv,
              "wp": np.ascontiguousarray(Wproj).astype(np.float16),
              "expb": expb_t, "ones": np.ones((128, 1), dtype=np.float16)}
    if use_bqk:
        common["bqk"] = bqk_t
    if use_bp:
        common["bp"] = np.ascontiguousarray(bp)
    in_maps = []
    for c in range(NCORES):
        m = dict(common)
        m["x"] = np.ascontiguousarray(x[c * BPC:(c + 1) * BPC])
        in_maps.append(m)
    return in_maps, use_bqk, use_bp


def run(inputs, trace=False, **run_kwargs):
    from concourse.bass_utils import run_bass_kernel_spmd

    in_maps, use_bqk, use_bp = _prepare(**inputs)
    key = (BPC, use_bqk, use_bp)
    if key not in _CACHE:
        _CACHE[key] = _build(*key)
    nc = _CACHE[key]
    res = run_bass_kernel_spmd(nc, in_maps, core_ids=list(range(NCORES)),
                               trace=trace, **run_kwargs)
    y = np.concatenate([res.results[c]["y"] for c in range(NCORES)], axis=0)
    return y, res


def kernel(**inputs):
    y, _ = run(inputs)
    return y


# revision 22
# speedup vs baseline: 1.1425x; 1.1425x over previous
# Self-contained Trainium2 Bass kernel for the LN->QKV->sparse-rel-pos-attention->proj block.
#
# Reference computation (B=128, N=256, DIM=512, H=12, KD=32, D=128):
#   xn   = LayerNorm(x) * gamma + beta
#   qkv  = xn @ Wqkv + bqkv ; split q,k,v per head
#   attn = softmax(q k^T / sqrt(KD) + biases[:, bias_idxs])
#   out  = (attn @ v) @ Wproj + bproj
#
# Strategy: pure data-parallel over batch across 8 NeuronCores (16 elems/core).
# Host folds: gamma/beta into Wqkv, 1/sqrt(KD) into Wq, v-bias into bproj,
# and expands exp(biases[:, bias_idxs]) so softmax(S+B) = expS*expB row-normalized.
# Device layouts avoid all transposes except the initial z -> z^T:
#   qk^T [feat, tok] and v [tok, feat] both come from matmuls against z^T;
#   S^T = k q^T has tokens-m on partitions so exp/Z/AV consume it directly;
#   AV gives O^T [head-dim, tok] which is exactly proj's stationary layout.
# All matmul operands are fp16 (PSUM accumulation stays fp32) so every matmul
# runs at the 2.4 GHz / 1-cycle-per-row stream rate. Scheduling keeps the PE
# queue dense: LN stats for pair p+1 are spread one-tile-at-a-time across
# pair p's attention groups, the Sqrt lands in the proj window (its two ACT
# table reloads hide behind the matmul-only transpose/QKV phase), pair p+1's
# transposes are emitted inside pair p's proj phase, and the S matmuls for
# group g+1 are issued before Z/AV of group g to cover the exp->mult latency.
# Softmax normalizer: Z^T = P^T-colsum via ones-matmul, reciprocal on DVE,
# DMA partition-broadcast, one fused DVE multiply on the AV output.

import numpy as np

B, N, DIM = 128, 256, 512
H, KD = 12, 32
D = 128
DH = D * H
RES = 16
EPS = 1e-5
NCORES = 8
BPC = B // NCORES

_CACHE = {}

# heads processed in strip-pure pairs: strips (h % 3) equal within each pair
HEAD_ORDER = [0, 3, 6, 9, 1, 4, 7, 10, 2, 5, 8, 11]


def _build(bpc, use_bqk, use_bp):
    from contextlib import ExitStack

    import concourse.bacc as bacc
    import concourse.tile as tile
    from concourse import mybir
    from concourse.masks import make_identity

    f32 = mybir.dt.float32
    f16 = mybir.dt.float16
    Alu = mybir.AluOpType
    Act = mybir.ActivationFunctionType

    nc = bacc.Bacc("TRN2", target_bir_lowering=False, debug=False,
                   num_devices=NCORES)

    x_d = nc.dram_tensor("x", [bpc, N, DIM], f32, kind="ExternalInput").ap()
    wqk_d = nc.dram_tensor("wqk", [DIM, 8 * 128], f16, kind="ExternalInput").ap()
    wv_d = nc.dram_tensor("wv", [DIM, DH], f16, kind="ExternalInput").ap()
    wp_d = nc.dram_tensor("wp", [DH, DIM], f16, kind="ExternalInput").ap()
    expb_d = nc.dram_tensor("expb", [128, 2, H, N], f16, kind="ExternalInput").ap()
    ones_d = nc.dram_tensor("ones", [128, 1], f16, kind="ExternalInput").ap()
    if use_bqk:
        bqk_d = nc.dram_tensor("bqk", [128, 8], f32, kind="ExternalInput").ap()
    if use_bp:
        bp_d = nc.dram_tensor("bp", [DIM], f32, kind="ExternalInput").ap()
    y_d = nc.dram_tensor("y", [bpc, N, DIM], f32, kind="ExternalOutput").ap()

    with tile.TileContext(nc) as tc, ExitStack() as ctx:
        consts = ctx.enter_context(tc.tile_pool(name="consts", bufs=1))
        sb_x = ctx.enter_context(tc.tile_pool(name="sb_x", bufs=2))
        sb_xn = ctx.enter_context(tc.tile_pool(name="sb_xn", bufs=8))
        sb_zT = ctx.enter_context(tc.tile_pool(name="sb_zT", bufs=2))
        sb_qkT = ctx.enter_context(tc.tile_pool(name="sb_qkT", bufs=1))
        sb_v = ctx.enter_context(tc.tile_pool(name="sb_v", bufs=2))
        sb_pt = ctx.enter_context(tc.tile_pool(name="sb_pt", bufs=4))
        sb_zb = ctx.enter_context(tc.tile_pool(name="sb_zb", bufs=2))
        sb_ot = ctx.enter_context(tc.tile_pool(name="sb_ot", bufs=2))
        sb_y = ctx.enter_context(tc.tile_pool(name="sb_y", bufs=2))
        sb_small = ctx.enter_context(tc.tile_pool(name="sb_small", bufs=3))
        ps_work = ctx.enter_context(tc.tile_pool(name="ps_work", bufs=2, space="PSUM"))
        ps_s = ctx.enter_context(tc.tile_pool(name="ps_s", bufs=3, space="PSUM"))
        ps_oty = ctx.enter_context(tc.tile_pool(name="ps_oty", bufs=3, space="PSUM"))
        dram = ctx.enter_context(tc.tile_pool(name="dram", bufs=2, space="DRAM"))

        # ---- constants.  The first pair's x tiles + LN chain are emitted
        # before the big weight DMAs so the transposes can start ~early;
        # weight DMA order follows first-use order (wqk, wv, expb, wp).
        ident = consts.tile([128, 128], f16)
        make_identity(nc, ident)
        ones_col = consts.tile([128, 1], f16)
        nc.sync.dma_start(out=ones_col, in_=ones_d)
        eps_t = consts.tile([128, 1], f32)
        nc.vector.memset(eps_t, EPS)
        if use_bqk:
            bqk_sb = consts.tile([128, 8], f32)
            nc.sync.dma_start(out=bqk_sb, in_=bqk_d)
        if use_bp:
            bp_sb = consts.tile([128, 1, DIM], f32)
            nc.sync.dma_start(out=bp_sb, in_=bp_d.partition_broadcast(128))

        assert bpc % 2 == 0
        npairs = bpc // 2

        def emit_stats_part(ep, i, st):
            # load one x tile and compute its bn stats (spread across the
            # attention groups so the DVE queue never blocks on a burst)
            if i == 0:
                st["mv"] = sb_small.tile([128, 2, 2, 2], f32, tag="mv",
                                         bufs=2, name="mv")
                st["x_ts"] = []
            el, tci = i // 2, i % 2
            x_t = sb_x.tile([128, DIM], f32, tag="x", bufs=8)
            nc.sync.dma_start(
                out=x_t,
                in_=x_d[2 * ep + el, tci * 128:(tci + 1) * 128, :])
            stats = sb_small.tile([128, 6], f32, tag="stats")
            nc.vector.bn_stats(stats, x_t)
            nc.vector.bn_aggr(st["mv"][:, el, tci, :], stats)
            st["x_ts"].append(x_t)

        def emit_norm(x_ts, mv):
            # single Sqrt per pair (table reloads between Sqrt and Exp are
            # ~1.3us each), then normalize into fp16 tiles
            sig = sb_small.tile([128, 2, 2], f32, tag="sig", bufs=2)
            nc.scalar.activation(sig, mv[:, :, :, 1], Act.Sqrt, bias=eps_t,
                                 scale=1.0)
            rsig = sb_small.tile([128, 2, 2], f32, tag="rsig", bufs=2)
            nc.vector.reciprocal(rsig, sig)
            xn_ts = []
            for el in range(2):
                for tci in range(2):
                    x_t = x_ts[2 * el + tci]
                    xn_t = sb_xn.tile([128, DIM], f16, tag="xn")
                    nc.vector.tensor_scalar(out=xn_t, in0=x_t,
                                            scalar1=mv[:, el, tci, 0:1],
                                            scalar2=rsig[:, el, tci:tci + 1],
                                            op0=Alu.subtract, op1=Alu.mult)
                    xn_ts.append(xn_t)
            return xn_ts

        def alloc_zT():
            return sb_zT.tile([128, 4, 2 * N], f16, tag="zT", name="zT_sb")

        def emit_transposes(zT_sb, xn_ts, el):
            # transposes time-share the S-matmul PSUM ring (fp16 view of the
            # f32 bank tile) — they run in the proj window when S is idle
            s_t = ps_s.tile([128, 512], f32, tag="s", name="s_ps")
            zT_ps = s_t.bitcast(f16).rearrange("p (a kc t) -> p a kc t",
                                               a=2, kc=4)
            for tci in range(2):
                xn_t = xn_ts[2 * el + tci]
                for kc in range(4):
                    nc.tensor.transpose(zT_ps[:, tci, kc, :],
                                        xn_t[:, kc * 128:(kc + 1) * 128],
                                        ident)
            for tci in range(2):
                off = el * N + tci * 128
                nc.scalar.activation(zT_sb[:, :, off:off + 128],
                                     zT_ps[:, tci, :, :], Act.Copy)

        st0 = {}
        for i in range(4):
            emit_stats_part(0, i, st0)
        xn_cur = emit_norm(st0["x_ts"], st0["mv"])

        # weight DMAs queue after the first pair's x so the LN/transpose
        # prologue is not stuck behind 5.5MB of weights
        wqk_sb = consts.tile([128, 4, 8 * 128], f16)
        nc.sync.dma_start(out=wqk_sb, in_=wqk_d.rearrange("(kc p) f -> p kc f", p=128))
        wv_sb = consts.tile([128, 4, DH], f16)
        nc.sync.dma_start(out=wv_sb, in_=wv_d.rearrange("(kc p) f -> p kc f", p=128))
        expb_sb = consts.tile([128, 2, H, N], f16)
        nc.sync.dma_start(out=expb_sb, in_=expb_d)
        wp_sb = consts.tile([128, H, DIM], f16)
        nc.sync.dma_start(out=wp_sb, in_=wp_d.rearrange("(h p) f -> p h f", p=128))

        zT_cur = alloc_zT()
        emit_transposes(zT_cur, xn_cur, 0)
        emit_transposes(zT_cur, xn_cur, 1)

        for ep in range(npairs):
            st_next = {}
            zT_sb = zT_cur

            # ---- qk^T = W'' ^T z^T   [feat, tok-pair].  Head h's q lives in
            # chunk h//3, its k in chunk 4 + h//3, both at 32-row strip h%3.
            # The S matmuls contract K=32 at partition base 32*(h%3); heads
            # are processed in strip-pure pairs because interleaving different
            # PE tile_positions on one PSUM bank hangs the device
            # (sem-separated bank reuse across strips is fine).
            qkT_sb = sb_qkT.tile([128, 8, 2 * N], f16, tag="qkT", bufs=1)
            for fc in range(8):
                qk_ps = ps_work.tile([128, 512], f32, tag="work")
                for kc in range(4):
                    nc.tensor.matmul(qk_ps,
                                     lhsT=wqk_sb[:, kc, fc * 128:(fc + 1) * 128],
                                     rhs=zT_sb[:, kc, :],
                                     start=(kc == 0), stop=(kc == 3))
                # GpSimd has no PSUM access path; split PSUM copies ACT/DVE
                if fc % 2 == 0:
                    nc.scalar.activation(qkT_sb[:, fc, :], qk_ps, Act.Copy)
                else:
                    nc.vector.tensor_copy(out=qkT_sb[:, fc, :], in_=qk_ps)
                if use_bqk:
                    nc.vector.tensor_scalar_add(
                        out=qkT_sb[:, fc, :], in0=qkT_sb[:, fc, :],
                        scalar1=bqk_sb[:, fc:fc + 1])

            xn_next = None
            zT_next = None

            # ---- v = z Wv  [tok 256, feat 1536], both elems up front ----
            v_tiles = []
            for el in range(2):
                etok = el * N
                v_sb = sb_v.tile([128, 2, DH], f16, tag="v", name="v_sb")
                v_tiles.append(v_sb)
                for mc in range(2):
                    for ns in range(3):
                        v_ps = ps_work.tile([128, 512], f32, tag="work")
                        for kc in range(4):
                            nc.tensor.matmul(
                                v_ps,
                                lhsT=zT_sb[:, kc,
                                           etok + mc * 128:etok + (mc + 1) * 128],
                                rhs=wv_sb[:, kc, ns * 512:(ns + 1) * 512],
                                start=(kc == 0), stop=(kc == 3))
                        if (mc * 3 + ns) % 2 == 0:
                            nc.scalar.activation(
                                v_sb[:, mc, ns * 512:(ns + 1) * 512], v_ps,
                                Act.Copy)
                        else:
                            nc.vector.tensor_copy(
                                out=v_sb[:, mc, ns * 512:(ns + 1) * 512],
                                in_=v_ps)

            # ---- attention: elem-fused head groups.  Each group handles one
            # head for BOTH elems (same k-strip => same PE tile_position on
            # the shared PSUM bank, so no pairing constraint).  s_ps columns
            # and pt/zb/ot_ps free dims are (el, n).
            ot_sb = sb_ot.tile([128, H, 2, N], f16, tag="ot")
            pt_tiles = [None] * H

            def emit_S(h):
                # S matmuls + exp + expB multiply for head h, both elems
                pt_sb = sb_pt.tile([128, 2, 2, N], f16, tag="pt",
                                   name="pt_sb")
                pt_tiles[h] = pt_sb
                c, base = h % 3, (h // 3) * KD
                for mc in range(2):
                    s_ps = ps_s.tile([128, 512], f32, tag="s", name="s_ps")
                    for el in range(2):
                        etok = el * N
                        nc.tensor.matmul(
                            s_ps[:, el * N:(el + 1) * N],
                            lhsT=qkT_sb[base:base + KD, 3 + c,
                                        etok + mc * 128:etok + (mc + 1) * 128],
                            rhs=qkT_sb[base:base + KD, c, etok:etok + N],
                            start=True, stop=True)
                    nc.scalar.activation(pt_sb[:, mc],
                                         s_ps.rearrange("p (a n) -> p a n",
                                                        a=2),
                                         Act.Exp)
                    # alternate the expB multiply between GpSimd and DVE
                    eng = nc.gpsimd if (h + mc) % 2 == 0 else nc.vector
                    eng.tensor_tensor(out=pt_sb[:, mc], in0=pt_sb[:, mc],
                                      in1=expb_sb[:, mc, h, :, :],
                                      op=Alu.mult)

            emit_S(0)
            for h in range(H):
                # issue the next group's S/exp/mult first so the PE has
                # work while group h's softmax chain drains
                if h + 1 < H:
                    emit_S(h + 1)
                pt_sb = pt_tiles[h]
                # Z[el, n] = sum_m P^T[m, (el n)]; one matmul per mc chunk
                zrow_ps = ps_oty.tile([128, 512], f32, tag="oty",
                                      name="zrow_ps")
                for mc in range(2):
                    nc.tensor.matmul(zrow_ps[0:1, :],
                                     lhsT=ones_col,
                                     rhs=pt_sb[:, mc, :, :].rearrange(
                                         "p a n -> p (a n)"),
                                     start=(mc == 0), stop=(mc == 1))
                zrecip_sb = sb_small.tile([1, 512], f32, tag="zrecip")
                nc.vector.reciprocal_approx_fast(out=zrecip_sb,
                                                 in_=zrow_ps[0:1, :])
                # partition-broadcast via DRAM roundtrip (step-0 partition
                # APs are only legal on DRAM sources)
                zscr = dram.tile([1, 512], f32, tag="zscr")
                nc.sync.dma_start(out=zscr, in_=zrecip_sb)
                zb_sb = sb_zb.tile([128, 2, N], f32, tag="zb")
                nc.sync.dma_start(out=zb_sb,
                                  in_=zscr[0, :].partition_broadcast(128))
                # O^T = v^T P^T  (normalized by zb afterwards)
                ot_ps = ps_oty.tile([128, 512], f32, tag="oty", name="ot_ps")
                for el in range(2):
                    for mc in range(2):
                        nc.tensor.matmul(
                            ot_ps[:, el * N:(el + 1) * N],
                            lhsT=v_tiles[el][:, mc, h * 128:(h + 1) * 128],
                            rhs=pt_sb[:, mc, el, :],
                            start=(mc == 0), stop=(mc == 1))
                nc.vector.tensor_tensor(
                    out=ot_sb[:, h, :, :],
                    in0=ot_ps.rearrange("p (a n) -> p a n", a=2),
                    in1=zb_sb, op=Alu.mult)
                # next pair's LN stats, one x-tile at a time so the DVE
                # queue never stalls the attention chain
                if ep + 1 < npairs and h in (2, 5, 8, 10):
                    emit_stats_part(ep + 1, (2, 5, 8, 10).index(h), st_next)

            # the Sqrt (and its two ACT table reloads) lands in the
            # proj/transpose window where the ACT engine runs no Exp
            if ep + 1 < npairs:
                xn_next = emit_norm(st_next["x_ts"], st_next["mv"])

            # ---- proj: y = O Wp ----  (next pair's transposes are
            # emitted between the proj chains to fill PE gaps)
            for el in range(2):
                e = 2 * ep + el
                for nci in range(2):
                    y_ps = ps_oty.tile([128, 512], f32, tag="oty", name="y_ps")
                    for slot in range(H):
                        nc.tensor.matmul(
                            y_ps,
                            lhsT=ot_sb[:, slot, el, nci * 128:(nci + 1) * 128],
                            rhs=wp_sb[:, slot, :],
                            start=(slot == 0), stop=(slot == H - 1))
                    yb_sb = sb_y.tile([128, DIM], f32, tag="yb")
                    if use_bp:
                        nc.vector.tensor_tensor(out=yb_sb, in0=y_ps,
                                                in1=bp_sb[:, 0, :], op=Alu.add)
                    else:
                        nc.scalar.activation(yb_sb, y_ps, Act.Copy)
                    nc.sync.dma_start(out=y_d[e, nci * 128:(nci + 1) * 128, :],
                                      in_=yb_sb)
                    if ep + 1 < npairs and nci == 1:
                        if el == 0:
                            zT_next = alloc_zT()
                        emit_transposes(zT_next, xn_next, el)

            xn_cur = xn_next
            zT_cur = zT_next

    nc.compile()
    return nc


def _prepare(x, gamma, beta, Wqkv, bqkv, Wproj, bproj, biases, bias_idxs):
    x = np.ascontiguousarray(np.asarray(x, dtype=np.float32))
    gamma = np.asarray(gamma, dtype=np.float32)
    beta = np.asarray(beta, dtype=np.float32)
    Wqkv = np.asarray(Wqkv, dtype=np.float32)
    bqkv = np.asarray(bqkv, dtype=np.float32)
    Wproj = np.asarray(Wproj, dtype=np.float32)
    bproj = np.asarray(bproj, dtype=np.float32)
    biases = np.asarray(biases, dtype=np.float32)
    bias_idxs = np.asarray(bias_idxs)

    s = np.float32(KD ** -0.5)
    Wg = Wqkv * gamma[:, None]
    bfull = beta @ Wqkv + bqkv
    Wr = Wg.reshape(DIM, H, 64 + D)
    br = bfull.reshape(H, 64 + D)
    # feature layout (see kernel comment): head h -> strip h%3; q in chunk
    # h//3, k in chunk 4 + h//3.
    wqk = np.zeros((DIM, 8, 128), dtype=np.float32)
    bqk = np.zeros((8, 128), dtype=np.float32)
    for h in range(H):
        qc, base = h // 3, (h % 3) * KD
        wqk[:, qc, base:base + KD] = Wr[:, h, 0:KD] * s
        wqk[:, 4 + qc, base:base + KD] = Wr[:, h, KD:2 * KD]
        bqk[qc, base:base + KD] = br[h, 0:KD] * s
        bqk[4 + qc, base:base + KD] = br[h, KD:2 * KD]
    wqk = np.ascontiguousarray(wqk.reshape(DIM, 8 * 128)).astype(np.float16)
    wv = np.ascontiguousarray(
        Wr[:, :, 2 * KD:].reshape(DIM, DH)).astype(np.float16)
    bv = br[:, 2 * KD:].reshape(DH)
    bp = bproj + bv @ Wproj
    expb = np.exp(biases[:, bias_idxs])  # [H, N, N]
    # head dim reordered to the kernel's strip-pure processing order
    expb_t = np.ascontiguousarray(
        expb[HEAD_ORDER].reshape(H, 2, 128, N).transpose(2, 1, 0, 3)
    ).astype(np.float16)

    use_bqk = bool(np.abs(bqk).max() > 0)
    use_bp = bool(np.abs(bp).max() > 0)
    bqk_t = np.ascontiguousarray(bqk.T)  # [128, 8]

    common = {"wqk": wqk, "wv": wv,
              "wp": np.ascontiguousarray(Wproj).astype(np.float16),
              "expb": expb_t, "ones": np.ones((128, 1), dtype=np.float16)}
    if use_bqk:
        common["bqk"] = bqk_t
    if use_bp:
        common["bp"] = np.ascontiguousarray(bp)
    in_maps = []
    for c in range(NCORES):
        m = dict(common)
        m["x"] = np.ascontiguousarray(x[c * BPC:(c + 1) * BPC])
        in_maps.append(m)
    return in_maps, use_bqk, use_bp


def run(inputs, trace=False, **run_kwargs):
    from concourse.bass_utils import run_bass_kernel_spmd

    in_maps, use_bqk, use_bp = _prepare(**inputs)
    key = (BPC, use_bqk, use_bp)
    if key not in _CACHE:
        _CACHE[key] = _build(*key)
    nc = _CACHE[key]
    res = run_bass_kernel_spmd(nc, in_maps, core_ids=list(range(NCORES)),
                               trace=trace, **run_kwargs)
    y = np.concatenate([res.results[c]["y"] for c in range(NCORES)], axis=0)
    return y, res


def kernel(**inputs):
    y, _ = run(inputs)
    return y


# revision 28
# speedup vs baseline: 1.1499x; 1.0065x over previous
# Self-contained Trainium2 Bass kernel for the LN->QKV->sparse-rel-pos-attention->proj block.
#
# Reference computation (B=128, N=256, DIM=512, H=12, KD=32, D=128):
#   xn   = LayerNorm(x) * gamma + beta
#   qkv  = xn @ Wqkv + bqkv ; split q,k,v per head
#   attn = softmax(q k^T / sqrt(KD) + biases[:, bias_idxs])
#   out  = (attn @ v) @ Wproj + bproj
#
# Strategy: pure data-parallel over batch across 8 NeuronCores (16 elems/core).
# Host folds: gamma/beta into Wqkv, 1/sqrt(KD) into Wq, v-bias into bproj,
# and expands exp(biases[:, bias_idxs]) so softmax(S+B) = expS*expB row-normalized.
# Device layouts avoid all transposes except the initial z -> z^T:
#   qk^T [feat, tok] and v [tok, feat] both come from matmuls against z^T;
#   S^T = k q^T has tokens-m on partitions so exp/Z/AV consume it directly;
#   AV gives O^T [head-dim, tok] which is exactly proj's stationary layout.
# All matmul operands are fp16 (PSUM accumulation stays fp32) so every matmul
# runs at the 2.4 GHz / 1-cycle-per-row stream rate. Scheduling keeps the PE
# queue dense: LN stats for pair p+1 are spread one-tile-at-a-time across
# pair p's attention groups, the Sqrt lands in the proj window (its two ACT
# table reloads hide behind the matmul-only transpose/QKV phase), pair p+1's
# transposes are emitted inside pair p's proj phase, and the S matmuls for
# group g+1 are issued before Z/AV of group g to cover the exp->mult latency.
# Softmax normalizer: Z^T = P^T-colsum via ones-matmul, reciprocal on DVE,
# DMA partition-broadcast, one fused DVE multiply on the AV output.

import numpy as np

B, N, DIM = 128, 256, 512
H, KD = 12, 32
D = 128
DH = D * H
RES = 16
EPS = 1e-5
NCORES = 8
BPC = B // NCORES

_CACHE = {}

# heads processed in strip-pure pairs: strips (h % 3) equal within each pair
HEAD_ORDER = [0, 3, 6, 9, 1, 4, 7, 10, 2, 5, 8, 11]


def _build(bpc, use_bqk, use_bp):
    from contextlib import ExitStack

    import concourse.bacc as bacc
    import concourse.tile as tile
    from concourse import mybir
    from concourse.masks import make_identity

    f32 = mybir.dt.float32
    f16 = mybir.dt.float16
    Alu = mybir.AluOpType
    Act = mybir.ActivationFunctionType

    nc = bacc.Bacc("TRN2", target_bir_lowering=False, debug=False,
                   num_devices=NCORES)

    x_d = nc.dram_tensor("x", [bpc, N, DIM], f32, kind="ExternalInput").ap()
    wqk_d = nc.dram_tensor("wqk", [DIM, 8 * 128], f16, kind="ExternalInput").ap()
    wv_d = nc.dram_tensor("wv", [DIM, DH], f16, kind="ExternalInput").ap()
    wp_d = nc.dram_tensor("wp", [DH, DIM], f16, kind="ExternalInput").ap()
    expb_d = nc.dram_tensor("expb", [128, 2, H, N], f16, kind="ExternalInput").ap()
    ones_d = nc.dram_tensor("ones", [128, 1], f16, kind="ExternalInput").ap()
    if use_bqk:
        bqk_d = nc.dram_tensor("bqk", [128, 8], f32, kind="ExternalInput").ap()
    if use_bp:
        bp_d = nc.dram_tensor("bp", [DIM], f32, kind="ExternalInput").ap()
    y_d = nc.dram_tensor("y", [bpc, N, DIM], f32, kind="ExternalOutput").ap()

    with tile.TileContext(nc) as tc, ExitStack() as ctx:
        consts = ctx.enter_context(tc.tile_pool(name="consts", bufs=1))
        sb_x = ctx.enter_context(tc.tile_pool(name="sb_x", bufs=2))
        sb_xn = ctx.enter_context(tc.tile_pool(name="sb_xn", bufs=8))
        sb_zT = ctx.enter_context(tc.tile_pool(name="sb_zT", bufs=2))
        sb_qkT = ctx.enter_context(tc.tile_pool(name="sb_qkT", bufs=1))
        sb_v = ctx.enter_context(tc.tile_pool(name="sb_v", bufs=2))
        sb_pt = ctx.enter_context(tc.tile_pool(name="sb_pt", bufs=4))
        sb_zb = ctx.enter_context(tc.tile_pool(name="sb_zb", bufs=2))
        sb_ot = ctx.enter_context(tc.tile_pool(name="sb_ot", bufs=2))
        sb_y = ctx.enter_context(tc.tile_pool(name="sb_y", bufs=2))
        sb_small = ctx.enter_context(tc.tile_pool(name="sb_small", bufs=3))
        ps_work = ctx.enter_context(tc.tile_pool(name="ps_work", bufs=2, space="PSUM"))
        ps_s = ctx.enter_context(tc.tile_pool(name="ps_s", bufs=2, space="PSUM"))
        ps_oty = ctx.enter_context(tc.tile_pool(name="ps_oty", bufs=3, space="PSUM"))
        ps_zt = ctx.enter_context(tc.tile_pool(name="ps_zt", bufs=1, space="PSUM"))
        dram = ctx.enter_context(tc.tile_pool(name="dram", bufs=2, space="DRAM"))

        # ---- constants ----
        wqk_sb = consts.tile([128, 4, 8 * 128], f16)
        nc.sync.dma_start(out=wqk_sb, in_=wqk_d.rearrange("(kc p) f -> p kc f", p=128))
        wv_sb = consts.tile([128, 4, DH], f16)
        nc.sync.dma_start(out=wv_sb, in_=wv_d.rearrange("(kc p) f -> p kc f", p=128))
        expb_sb = consts.tile([128, 2, H, N], f16)
        nc.sync.dma_start(out=expb_sb, in_=expb_d)
        wp_sb = consts.tile([128, H, DIM], f16)
        nc.sync.dma_start(out=wp_sb, in_=wp_d.rearrange("(h p) f -> p h f", p=128))
        ident = consts.tile([128, 128], f16)
        make_identity(nc, ident)
        ones_col = consts.tile([128, 1], f16)
        nc.sync.dma_start(out=ones_col, in_=ones_d)
        eps_t = consts.tile([128, 1], f32)
        nc.vector.memset(eps_t, EPS)
        if use_bqk:
            bqk_sb = consts.tile([128, 8], f32)
            nc.sync.dma_start(out=bqk_sb, in_=bqk_d)
        if use_bp:
            bp_sb = consts.tile([128, 1, DIM], f32)
            nc.sync.dma_start(out=bp_sb, in_=bp_d.partition_broadcast(128))

        assert bpc % 2 == 0
        npairs = bpc // 2

        def emit_stats_part(ep, i, st):
            # load one x tile and compute its bn stats (spread across the
            # attention groups so the DVE queue never blocks on a burst)
            if i == 0:
                st["mv"] = sb_small.tile([128, 2, 2, 2], f32, tag="mv",
                                         bufs=2, name="mv")
                st["x_ts"] = []
            el, tci = i // 2, i % 2
            x_t = sb_x.tile([128, DIM], f32, tag="x", bufs=8)
            nc.sync.dma_start(
                out=x_t,
                in_=x_d[2 * ep + el, tci * 128:(tci + 1) * 128, :])
            stats = sb_small.tile([128, 6], f32, tag="stats")
            nc.vector.bn_stats(stats, x_t)
            nc.vector.bn_aggr(st["mv"][:, el, tci, :], stats)
            st["x_ts"].append(x_t)

        def emit_norm(x_ts, mv):
            # single Sqrt per pair (table reloads between Sqrt and Exp are
            # ~1.3us each), then normalize into fp16 tiles
            sig = sb_small.tile([128, 2, 2], f32, tag="sig", bufs=2)
            nc.scalar.activation(sig, mv[:, :, :, 1], Act.Sqrt, bias=eps_t,
                                 scale=1.0)
            rsig = sb_small.tile([128, 2, 2], f32, tag="rsig", bufs=2)
            nc.vector.reciprocal(rsig, sig)
            xn_ts = []
            for el in range(2):
                for tci in range(2):
                    x_t = x_ts[2 * el + tci]
                    xn_t = sb_xn.tile([128, DIM], f16, tag="xn")
                    nc.vector.tensor_scalar(out=xn_t, in0=x_t,
                                            scalar1=mv[:, el, tci, 0:1],
                                            scalar2=rsig[:, el, tci:tci + 1],
                                            op0=Alu.subtract, op1=Alu.mult)
                    xn_ts.append(xn_t)
            return xn_ts

        def alloc_zT():
            return sb_zT.tile([128, 4, 2 * N], f16, tag="zT", name="zT_sb")

        def emit_transposes(zT_sb, xn_ts, el):
            zT_ps = ps_zt.tile([128, 2, 4, 128], f16, tag="zTps", name="zT_ps")
            for tci in range(2):
                xn_t = xn_ts[2 * el + tci]
                for kc in range(4):
                    nc.tensor.transpose(zT_ps[:, tci, kc, :],
                                        xn_t[:, kc * 128:(kc + 1) * 128],
                                        ident)
            for tci in range(2):
                off = el * N + tci * 128
                nc.scalar.activation(zT_sb[:, :, off:off + 128],
                                     zT_ps[:, tci, :, :], Act.Copy)

        st0 = {}
        for i in range(4):
            emit_stats_part(0, i, st0)
        xn_cur = emit_norm(st0["x_ts"], st0["mv"])
        zT_cur = alloc_zT()
        emit_transposes(zT_cur, xn_cur, 0)
        emit_transposes(zT_cur, xn_cur, 1)

        for ep in range(npairs):
            st_next = {}
            zT_sb = zT_cur

            # ---- qk^T = W'' ^T z^T   [feat, tok-pair].  Head h's q lives in
            # chunk h//3, its k in chunk 4 + h//3, both at 32-row strip h%3.
            # The S matmuls contract K=32 at partition base 32*(h%3); heads
            # are processed in strip-pure pairs because interleaving different
            # PE tile_positions on one PSUM bank hangs the device
            # (sem-separated bank reuse across strips is fine).
            qkT_sb = sb_qkT.tile([128, 8, 2 * N], f16, tag="qkT", bufs=1)
            for fc in range(8):
                qk_ps = ps_work.tile([128, 512], f32, tag="work")
                for kc in range(4):
                    nc.tensor.matmul(qk_ps,
                                     lhsT=wqk_sb[:, kc, fc * 128:(fc + 1) * 128],
                                     rhs=zT_sb[:, kc, :],
                                     start=(kc == 0), stop=(kc == 3))
                # GpSimd has no PSUM access path; split PSUM copies ACT/DVE
                if fc % 2 == 0:
                    nc.scalar.activation(qkT_sb[:, fc, :], qk_ps, Act.Copy)
                else:
                    nc.vector.tensor_copy(out=qkT_sb[:, fc, :], in_=qk_ps)
                if use_bqk:
                    nc.vector.tensor_scalar_add(
                        out=qkT_sb[:, fc, :], in0=qkT_sb[:, fc, :],
                        scalar1=bqk_sb[:, fc:fc + 1])

            xn_next = None
            zT_next = None

            for el in range(2):
                e = 2 * ep + el
                etok = el * N
                # ---- v = z Wv   [tok 256, feat 1536] ----
                v_sb = sb_v.tile([128, 2, DH], f16, tag="v")
                for mc in range(2):
                    for ns in range(3):
                        v_ps = ps_work.tile([128, 512], f32, tag="work")
                        for kc in range(4):
                            nc.tensor.matmul(
                                v_ps,
                                lhsT=zT_sb[:, kc,
                                           etok + mc * 128:etok + (mc + 1) * 128],
                                rhs=wv_sb[:, kc, ns * 512:(ns + 1) * 512],
                                start=(kc == 0), stop=(kc == 3))
                        if (mc * 3 + ns) % 2 == 0:
                            nc.scalar.activation(
                                v_sb[:, mc, ns * 512:(ns + 1) * 512], v_ps,
                                Act.Copy)
                        else:
                            nc.vector.tensor_copy(
                                out=v_sb[:, mc, ns * 512:(ns + 1) * 512],
                                in_=v_ps)

                # ---- attention, strip-pure head pairs ----
                # slot 2g+hl in pt/ot/expb corresponds to HEAD_ORDER[2g+hl]
                ot_sb = sb_ot.tile([128, H, N], f16, tag="ot")
                pt_tiles = [None] * 6

                def emit_S(g):
                    # S matmuls + exp + expB multiply for group g
                    pt_sb = sb_pt.tile([128, 2, 2, N], f16, tag="pt",
                                       name="pt_sb")
                    pt_tiles[g] = pt_sb
                    for mc in range(2):
                        s_ps = ps_s.tile([128, 512], f32, tag="s", name="s_ps")
                        for hl in range(2):
                            h = HEAD_ORDER[2 * g + hl]
                            qc = h // 3
                            base = (h % 3) * KD
                            nc.tensor.matmul(
                                s_ps[:, hl * N:(hl + 1) * N],
                                lhsT=qkT_sb[base:base + KD, 4 + qc,
                                            etok + mc * 128:etok + (mc + 1) * 128],
                                rhs=qkT_sb[base:base + KD, qc, etok:etok + N],
                                start=True, stop=True)
                        nc.scalar.activation(pt_sb[:, mc],
                                             s_ps.rearrange("p (a n) -> p a n",
                                                            a=2),
                                             Act.Exp)
                        # alternate the expB multiply between GpSimd and DVE
                        eng = nc.gpsimd if (g + mc) % 2 == 0 else nc.vector
                        eng.tensor_tensor(out=pt_sb[:, mc], in0=pt_sb[:, mc],
                                          in1=expb_sb[:, mc, 2 * g:2 * g + 2, :],
                                          op=Alu.mult)

                emit_S(0)
                for g in range(6):
                    # issue the next group's S/exp/mult first so the PE has
                    # work while group g's softmax chain drains
                    if g + 1 < 6:
                        emit_S(g + 1)
                    pt_sb = pt_tiles[g]
                    # Z[hl, n] = sum_m P^T[m, n]; one matmul per mc chunk
                    zrow_ps = ps_oty.tile([128, 512], f32, tag="oty",
                                          name="zrow_ps")
                    for mc in range(2):
                        nc.tensor.matmul(zrow_ps[0:1, :],
                                         lhsT=ones_col,
                                         rhs=pt_sb[:, mc, :, :].rearrange(
                                             "p a n -> p (a n)"),
                                         start=(mc == 0), stop=(mc == 1))
                    zrecip_sb = sb_small.tile([1, 512], f32, tag="zrecip")
                    nc.vector.reciprocal_approx_fast(out=zrecip_sb,
                                                     in_=zrow_ps[0:1, :])
                    # partition-broadcast via DRAM roundtrip (step-0 partition
                    # APs are only legal on DRAM sources)
                    zscr = dram.tile([1, 512], f32, tag="zscr")
                    nc.sync.dma_start(out=zscr, in_=zrecip_sb)
                    zb_sb = sb_zb.tile([128, 2, N], f32, tag="zb")
                    nc.sync.dma_start(out=zb_sb,
                                      in_=zscr[0, :].partition_broadcast(128))
                    # O^T = v^T P^T  (normalized by zb afterwards)
                    ot_ps = ps_oty.tile([128, 512], f32, tag="oty",
                                        name="ot_ps")
                    for hl in range(2):
                        h = HEAD_ORDER[2 * g + hl]
                        for mc in range(2):
                            nc.tensor.matmul(
                                ot_ps[:, hl * N:(hl + 1) * N],
                                lhsT=v_sb[:, mc, h * 128:(h + 1) * 128],
                                rhs=pt_sb[:, mc, hl, :],
                                start=(mc == 0), stop=(mc == 1))
                    nc.vector.tensor_tensor(
                        out=ot_sb[:, 2 * g:2 * g + 2, :],
                        in0=ot_ps.rearrange("p (a n) -> p a n", a=2),
                        in1=zb_sb, op=Alu.mult)
                    # next pair's LN stats, one x-tile at a time so the DVE
                    # queue never stalls the attention chain
                    if ep + 1 < npairs:
                        if el == 0 and g % 2 == 1:
                            emit_stats_part(ep + 1, g // 2, st_next)
                        elif el == 1 and g == 1:
                            emit_stats_part(ep + 1, 3, st_next)

                # the Sqrt (and its two ACT table reloads) lands in the
                # proj/transpose window where the ACT engine runs no Exp
                if el == 1 and ep + 1 < npairs:
                    xn_next = emit_norm(st_next["x_ts"], st_next["mv"])

                # ---- proj: y = O Wp ----  (next pair's transposes are
                # emitted between the two proj chains to fill PE gaps)
                for nci in range(2):
                    y_ps = ps_oty.tile([128, 512], f32, tag="oty", name="y_ps")
                    for slot in range(H):
                        nc.tensor.matmul(y_ps,
                                         lhsT=ot_sb[:, slot, nci * 128:(nci + 1) * 128],
                                         rhs=wp_sb[:, HEAD_ORDER[slot], :],
                                         start=(slot == 0), stop=(slot == H - 1))
                    yb_sb = sb_y.tile([128, DIM], f32, tag="yb")
                    if use_bp:
                        nc.vector.tensor_tensor(out=yb_sb, in0=y_ps,
                                                in1=bp_sb[:, 0, :], op=Alu.add)
                    else:
                        nc.scalar.activation(yb_sb, y_ps, Act.Copy)
                    nc.sync.dma_start(out=y_d[e, nci * 128:(nci + 1) * 128, :],
                                      in_=yb_sb)
                    if el == 1 and ep + 1 < npairs:
                        if nci == 0:
                            zT_next = alloc_zT()
                        emit_transposes(zT_next, xn_next, nci)

            xn_cur = xn_next
            zT_cur = zT_next

    nc.compile()
    return nc


def _prepare(x, gamma, beta, Wqkv, bqkv, Wproj, bproj, biases, bias_idxs):
    x = np.ascontiguousarray(np.asarray(x, dtype=np.float32))
    gamma = np.asarray(gamma, dtype=np.float32)
    beta = np.asarray(beta, dtype=np.float32)
    Wqkv = np.asarray(Wqkv, dtype=np.float32)
    bqkv = np.asarray(bqkv, dtype=np.float32)
    Wproj = np.asarray(Wproj, dtype=np.float32)
    bproj = np.asarray(bproj, dtype=np.float32)
    biases = np.asarray(biases, dtype=np.float32)
    bias_idxs = np.asarray(bias_idxs)

    s = np.float32(KD ** -0.5)
    Wg = Wqkv * gamma[:, None]
    bfull = beta @ Wqkv + bqkv
    Wr = Wg.reshape(DIM, H, 64 + D)
    br = bfull.reshape(H, 64 + D)
    # feature layout (see kernel comment): head h -> strip h%3; q in chunk
    # h//3, k in chunk 4 + h//3.
    wqk = np.zeros((DIM, 8, 128), dtype=np.float32)
    bqk = np.zeros((8, 128), dtype=np.float32)
    for h in range(H):
        qc, base = h // 3, (h % 3) * KD
        wqk[:, qc, base:base + KD] = Wr[:, h, 0:KD] * s
        wqk[:, 4 + qc, base:base + KD] = Wr[:, h, KD:2 * KD]
        bqk[qc, base:base + KD] = br[h, 0:KD] * s
        bqk[4 + qc, base:base + KD] = br[h, KD:2 * KD]
    wqk = np.ascontiguousarray(wqk.reshape(DIM, 8 * 128)).astype(np.float16)
    wv = np.ascontiguousarray(
        Wr[:, :, 2 * KD:].reshape(DIM, DH)).astype(np.float16)
    bv = br[:, 2 * KD:].reshape(DH)
    bp = bproj + bv @ Wproj
    expb = np.exp(biases[:, bias_idxs])  # [H, N, N]
    # head dim reordered to the kernel's strip-pure processing order
    expb_t = np.ascontiguousarray(
        expb[HEAD_ORDER].reshape(H, 2, 128, N).transpose(2, 1, 0, 3)
    ).astype(np.float16)

    use_bqk = bool(np.abs(bqk).max() > 0)
    use_bp = bool(np.abs(bp).max() > 0)
    bqk_t = np.ascontiguousarray(bqk.T)  # [128, 8]

    common = {"wqk": wqk, "wv": wv,
              "wp": np.ascontiguousarray(Wproj).astype(np.float16),
              "expb": expb_t, "ones": np.ones((128, 1), dtype=np.float16)}
    if use_bqk:
        common["bqk"] = bqk_t
    if use_bp:
        common["bp"] = np.ascontiguousarray(bp)
    in_maps = []
    for c in range(NCORES):
        m = dict(common)
        m["x"] = np.ascontiguousarray(x[c * BPC:(c + 1) * BPC])
        in_maps.append(m)
    return in_maps, use_bqk, use_bp


def run(inputs, trace=False, **run_kwargs):
    from concourse.bass_utils import run_bass_kernel_spmd

    in_maps, use_bqk, use_bp = _prepare(**inputs)
    key = (BPC, use_bqk, use_bp)
    if key not in _CACHE:
        _CACHE[key] = _build(*key)
    nc = _CACHE[key]
    res = run_bass_kernel_spmd(nc, in_maps, core_ids=list(range(NCORES)),
                               trace=trace, **run_kwargs)
    y = np.concatenate([res.results[c]["y"] for c in range(NCORES)], axis=0)
    return y, res


def kernel(**inputs):
    y, _ = run(inputs)
    return y


# revision 35
# speedup vs baseline: 1.3502x; 1.1741x over previous
# Self-contained Trainium2 Bass kernel for the LN->QKV->sparse-rel-pos-attention->proj block.
#
# Reference computation (B=128, N=256, DIM=512, H=12, KD=32, D=128):
#   xn   = LayerNorm(x) * gamma + beta
#   qkv  = xn @ Wqkv + bqkv ; split q,k,v per head
#   attn = softmax(q k^T / sqrt(KD) + biases[:, bias_idxs])
#   out  = (attn @ v) @ Wproj + bproj
#
# Strategy: pure data-parallel over batch across 8 NeuronCores (16 elems/core).
# Host folds: gamma/beta into Wqkv, 1/sqrt(KD) into Wq, v-bias into bproj,
# and expands exp(biases[:, bias_idxs]) so softmax(S+B) = expS*expB row-normalized.
# Device layouts avoid all transposes except the initial z -> z^T:
#   qk^T [feat, tok] and v [tok, feat] both come from matmuls against z^T;
#   S^T = k q^T has tokens-m on partitions so exp/Z/AV consume it directly;
#   AV gives O^T [head-dim, tok] which is exactly proj's stationary layout.
# All matmul operands are fp16 (PSUM accumulation stays fp32) so every matmul
# runs at the 2.4 GHz / 1-cycle-per-row stream rate. Scheduling keeps the PE
# queue dense: LN stats for pair p+1 are spread one-tile-at-a-time across
# pair p's attention groups, the Sqrt lands in the proj window (its two ACT
# table reloads hide behind the matmul-only transpose/QKV phase), pair p+1's
# transposes are emitted inside pair p's proj phase, and the S matmuls for
# group g+1 are issued before Z/AV of group g to cover the exp->mult latency.
# Softmax normalizer: Z^T = P^T-colsum via ones-matmul, reciprocal on DVE,
# DMA partition-broadcast, one fused DVE multiply on the AV output.

import numpy as np

B, N, DIM = 128, 256, 512
H, KD = 12, 32
D = 128
DH = D * H
RES = 16
EPS = 1e-5
NCORES = 8
BPC = B // NCORES

_CACHE = {}

# heads processed in strip-pure pairs: strips (h % 3) equal within each pair
HEAD_ORDER = [0, 3, 6, 9, 1, 4, 7, 10, 2, 5, 8, 11]


def _build(bpc, use_bqk, use_bp):
    from contextlib import ExitStack

    import concourse.bacc as bacc
    import concourse.tile as tile
    from concourse import mybir
    from concourse.masks import make_identity

    f32 = mybir.dt.float32
    f16 = mybir.dt.float16
    Alu = mybir.AluOpType
    Act = mybir.ActivationFunctionType

    nc = bacc.Bacc("TRN2", target_bir_lowering=False, debug=False,
                   num_devices=NCORES)

    x_d = nc.dram_tensor("x", [bpc, N, DIM], f32, kind="ExternalInput").ap()
    wqk_d = nc.dram_tensor("wqk", [DIM, 8 * 128], f16, kind="ExternalInput").ap()
    wv_d = nc.dram_tensor("wv", [DIM, DH], f16, kind="ExternalInput").ap()
    wp_d = nc.dram_tensor("wp", [DH, DIM], f16, kind="ExternalInput").ap()
    expb_d = nc.dram_tensor("expb", [128, 2, H, N], f16, kind="ExternalInput").ap()
    ones_d = nc.dram_tensor("ones", [128, 1], f16, kind="ExternalInput").ap()
    if use_bqk:
        bqk_d = nc.dram_tensor("bqk", [128, 8], f32, kind="ExternalInput").ap()
    if use_bp:
        bp_d = nc.dram_tensor("bp", [DIM], f32, kind="ExternalInput").ap()
    y_d = nc.dram_tensor("y", [bpc, N, DIM], f32, kind="ExternalOutput").ap()

    with tile.TileContext(nc) as tc, ExitStack() as ctx:
        consts = ctx.enter_context(tc.tile_pool(name="consts", bufs=1))
        sb_x = ctx.enter_context(tc.tile_pool(name="sb_x", bufs=2))
        sb_xn = ctx.enter_context(tc.tile_pool(name="sb_xn", bufs=8))
        sb_zT = ctx.enter_context(tc.tile_pool(name="sb_zT", bufs=2))
        sb_qkT = ctx.enter_context(tc.tile_pool(name="sb_qkT", bufs=1))
        sb_v = ctx.enter_context(tc.tile_pool(name="sb_v", bufs=2))
        sb_pt = ctx.enter_context(tc.tile_pool(name="sb_pt", bufs=4))
        sb_zb = ctx.enter_context(tc.tile_pool(name="sb_zb", bufs=2))
        sb_ot = ctx.enter_context(tc.tile_pool(name="sb_ot", bufs=2))
        sb_y = ctx.enter_context(tc.tile_pool(name="sb_y", bufs=2))
        sb_small = ctx.enter_context(tc.tile_pool(name="sb_small", bufs=3))
        ps_work = ctx.enter_context(tc.tile_pool(name="ps_work", bufs=2, space="PSUM"))
        ps_s = ctx.enter_context(tc.tile_pool(name="ps_s", bufs=3, space="PSUM"))
        ps_oty = ctx.enter_context(tc.tile_pool(name="ps_oty", bufs=3, space="PSUM"))
        dram = ctx.enter_context(tc.tile_pool(name="dram", bufs=2, space="DRAM"))

        eps_t = consts.tile([128, 1], f32)
        nc.vector.memset(eps_t, EPS)

        assert bpc % 2 == 0
        npairs = bpc // 2

        def emit_stats_part(ep, i, st):
            # load one x tile and compute its bn stats (spread across the
            # attention groups so the DVE queue never blocks on a burst)
            if i == 0:
                st["mv"] = sb_small.tile([128, 2, 2, 2], f32, tag="mv",
                                         bufs=2, name="mv")
                st["x_ts"] = []
            el, tci = i // 2, i % 2
            x_t = sb_x.tile([128, DIM], f32, tag="x", bufs=8)
            nc.sync.dma_start(
                out=x_t,
                in_=x_d[2 * ep + el, tci * 128:(tci + 1) * 128, :])
            stats = sb_small.tile([128, 6], f32, tag="stats")
            nc.vector.bn_stats(stats, x_t)
            nc.vector.bn_aggr(st["mv"][:, el, tci, :], stats)
            st["x_ts"].append(x_t)

        def emit_norm(x_ts, mv):
            # single Sqrt per pair (table reloads between Sqrt and Exp are
            # ~1.3us each), then normalize into fp16 tiles
            sig = sb_small.tile([128, 2, 2], f32, tag="sig", bufs=2)
            nc.scalar.activation(sig, mv[:, :, :, 1], Act.Sqrt, bias=eps_t,
                                 scale=1.0)
            rsig = sb_small.tile([128, 2, 2], f32, tag="rsig", bufs=2)
            nc.vector.reciprocal(rsig, sig)
            xn_ts = []
            for el in range(2):
                for tci in range(2):
                    x_t = x_ts[2 * el + tci]
                    xn_t = sb_xn.tile([128, DIM], f16, tag="xn")
                    nc.vector.tensor_scalar(out=xn_t, in0=x_t,
                                            scalar1=mv[:, el, tci, 0:1],
                                            scalar2=rsig[:, el, tci:tci + 1],
                                            op0=Alu.subtract, op1=Alu.mult)
                    xn_ts.append(xn_t)
            return xn_ts

        def alloc_zT():
            return sb_zT.tile([128, 4, 2 * N], f16, tag="zT", name="zT_sb")

        def emit_transposes(zT_sb, xn_ts, el):
            # transposes time-share the S-matmul PSUM ring (fp16 view of the
            # f32 bank tile) — they run in the proj window when S is idle
            s_t = ps_s.tile([128, 512], f32, tag="s", name="s_ps")
            zT_ps = s_t.bitcast(f16).rearrange("p (a kc t) -> p a kc t",
                                               a=2, kc=4)
            for tci in range(2):
                xn_t = xn_ts[2 * el + tci]
                for kc in range(4):
                    nc.tensor.transpose(zT_ps[:, tci, kc, :],
                                        xn_t[:, kc * 128:(kc + 1) * 128],
                                        ident)
            for tci in range(2):
                off = el * N + tci * 128
                nc.scalar.activation(zT_sb[:, :, off:off + 128],
                                     zT_ps[:, tci, :, :], Act.Copy)

        # pair-0's x loads + LN stats go first so the transposes are not
        # stuck behind 5.5MB of weight DMAs; weights follow in use order
        st0 = {}
        for i in range(4):
            emit_stats_part(0, i, st0)
        xn_cur = emit_norm(st0["x_ts"], st0["mv"])

        wqk_sb = consts.tile([128, 4, 8 * 128], f16)
        nc.sync.dma_start(out=wqk_sb, in_=wqk_d.rearrange("(kc p) f -> p kc f", p=128))
        wv_sb = consts.tile([128, 4, DH], f16)
        nc.sync.dma_start(out=wv_sb, in_=wv_d.rearrange("(kc p) f -> p kc f", p=128))
        expb_sb = consts.tile([128, 2, H, N], f16)
        nc.sync.dma_start(out=expb_sb, in_=expb_d)
        wp_sb = consts.tile([128, H, DIM], f16)
        nc.sync.dma_start(out=wp_sb, in_=wp_d.rearrange("(h p) f -> p h f", p=128))
        ident = consts.tile([128, 128], f16)
        make_identity(nc, ident)
        ones_col = consts.tile([128, 1], f16)
        nc.sync.dma_start(out=ones_col, in_=ones_d)
        if use_bqk:
            bqk_sb = consts.tile([128, 8], f32)
            nc.sync.dma_start(out=bqk_sb, in_=bqk_d)
        if use_bp:
            bp_sb = consts.tile([128, 1, DIM], f32)
            nc.sync.dma_start(out=bp_sb, in_=bp_d.partition_broadcast(128))

        zT_cur = alloc_zT()
        emit_transposes(zT_cur, xn_cur, 0)
        emit_transposes(zT_cur, xn_cur, 1)

        for ep in range(npairs):
            st_next = {}
            zT_sb = zT_cur

            # ---- qk^T = W'' ^T z^T   [feat, tok-pair].  Head h's q lives in
            # chunk h//3, its k in chunk 4 + h//3, both at 32-row strip h%3.
            # The S matmuls contract K=32 at partition base 32*(h%3); heads
            # are processed in strip-pure pairs because interleaving different
            # PE tile_positions on one PSUM bank hangs the device
            # (sem-separated bank reuse across strips is fine).
            qkT_sb = sb_qkT.tile([128, 8, 2 * N], f16, tag="qkT", bufs=1)
            for fc in range(8):
                qk_ps = ps_work.tile([128, 512], f32, tag="work")
                for kc in range(4):
                    nc.tensor.matmul(qk_ps,
                                     lhsT=wqk_sb[:, kc, fc * 128:(fc + 1) * 128],
                                     rhs=zT_sb[:, kc, :],
                                     start=(kc == 0), stop=(kc == 3))
                # GpSimd has no PSUM access path; split PSUM copies ACT/DVE
                if fc % 2 == 0:
                    nc.scalar.activation(qkT_sb[:, fc, :], qk_ps, Act.Copy)
                else:
                    nc.vector.tensor_copy(out=qkT_sb[:, fc, :], in_=qk_ps)
                if use_bqk:
                    nc.vector.tensor_scalar_add(
                        out=qkT_sb[:, fc, :], in0=qkT_sb[:, fc, :],
                        scalar1=bqk_sb[:, fc:fc + 1])

            xn_next = None
            zT_next = None

            for el in range(2):
                e = 2 * ep + el
                etok = el * N
                # ---- v = z Wv   [tok 256, feat 1536] ----
                v_sb = sb_v.tile([128, 2, DH], f16, tag="v")
                for mc in range(2):
                    for ns in range(3):
                        v_ps = ps_work.tile([128, 512], f32, tag="work")
                        for kc in range(4):
                            nc.tensor.matmul(
                                v_ps,
                                lhsT=zT_sb[:, kc,
                                           etok + mc * 128:etok + (mc + 1) * 128],
                                rhs=wv_sb[:, kc, ns * 512:(ns + 1) * 512],
                                start=(kc == 0), stop=(kc == 3))
                        if (mc * 3 + ns) % 2 == 0:
                            nc.scalar.activation(
                                v_sb[:, mc, ns * 512:(ns + 1) * 512], v_ps,
                                Act.Copy)
                        else:
                            nc.vector.tensor_copy(
                                out=v_sb[:, mc, ns * 512:(ns + 1) * 512],
                                in_=v_ps)

                # ---- attention, strip-pure head pairs ----
                # slot 2g+hl in pt/ot/expb corresponds to HEAD_ORDER[2g+hl]
                ot_sb = sb_ot.tile([128, H, N], f16, tag="ot")
                pt_tiles = [None] * 6

                def emit_S(g):
                    # S matmuls + exp + expB multiply for group g
                    pt_sb = sb_pt.tile([128, 2, 2, N], f16, tag="pt",
                                       name="pt_sb")
                    pt_tiles[g] = pt_sb
                    for mc in range(2):
                        s_ps = ps_s.tile([128, 512], f32, tag="s", name="s_ps")
                        for hl in range(2):
                            h = HEAD_ORDER[2 * g + hl]
                            qc = h // 3
                            base = (h % 3) * KD
                            nc.tensor.matmul(
                                s_ps[:, hl * N:(hl + 1) * N],
                                lhsT=qkT_sb[base:base + KD, 4 + qc,
                                            etok + mc * 128:etok + (mc + 1) * 128],
                                rhs=qkT_sb[base:base + KD, qc, etok:etok + N],
                                start=True, stop=True)
                        nc.scalar.activation(pt_sb[:, mc],
                                             s_ps.rearrange("p (a n) -> p a n",
                                                            a=2),
                                             Act.Exp)
                        # mc0's multiply has the full S(mc1)+lookahead window
                        # to drain, so it can take the slow GpSimd; mc1 is the
                        # critical tail and always gets the fast DVE
                        eng = nc.gpsimd if mc == 0 else nc.vector
                        eng.tensor_tensor(out=pt_sb[:, mc], in0=pt_sb[:, mc],
                                          in1=expb_sb[:, mc, 2 * g:2 * g + 2, :],
                                          op=Alu.mult)

                emit_S(0)
                for g in range(6):
                    # issue the next group's S/exp/mult first so the PE has
                    # work while group g's softmax chain drains
                    if g + 1 < 6:
                        emit_S(g + 1)
                    pt_sb = pt_tiles[g]
                    # Z[hl, n] = sum_m P^T[m, n]; one matmul per mc chunk
                    zrow_ps = ps_oty.tile([128, 512], f32, tag="oty",
                                          name="zrow_ps")
                    for mc in range(2):
                        nc.tensor.matmul(zrow_ps[0:1, :],
                                         lhsT=ones_col,
                                         rhs=pt_sb[:, mc, :, :].rearrange(
                                             "p a n -> p (a n)"),
                                         start=(mc == 0), stop=(mc == 1))
                    zrecip_sb = sb_small.tile([1, 512], f32, tag="zrecip")
                    nc.vector.reciprocal_approx_fast(out=zrecip_sb,
                                                     in_=zrow_ps[0:1, :])
                    # partition-broadcast via DRAM roundtrip (step-0 partition
                    # APs are only legal on DRAM sources)
                    zscr = dram.tile([1, 512], f32, tag="zscr")
                    nc.sync.dma_start(out=zscr, in_=zrecip_sb)
                    zb_sb = sb_zb.tile([128, 2, N], f32, tag="zb")
                    nc.sync.dma_start(out=zb_sb,
                                      in_=zscr[0, :].partition_broadcast(128))
                    # O^T = v^T P^T  (normalized by zb afterwards)
                    ot_ps = ps_oty.tile([128, 512], f32, tag="oty",
                                        name="ot_ps")
                    for hl in range(2):
                        h = HEAD_ORDER[2 * g + hl]
                        for mc in range(2):
                            nc.tensor.matmul(
                                ot_ps[:, hl * N:(hl + 1) * N],
                                lhsT=v_sb[:, mc, h * 128:(h + 1) * 128],
                                rhs=pt_sb[:, mc, hl, :],
                                start=(mc == 0), stop=(mc == 1))
                    nc.vector.tensor_tensor(
                        out=ot_sb[:, 2 * g:2 * g + 2, :],
                        in0=ot_ps.rearrange("p (a n) -> p a n", a=2),
                        in1=zb_sb, op=Alu.mult)
                    # next pair's LN stats, one x-tile at a time so the DVE
                    # queue never stalls the attention chain
                    if ep + 1 < npairs:
                        if el == 0 and g % 2 == 1:
                            emit_stats_part(ep + 1, g // 2, st_next)
                        elif el == 1 and g == 1:
                            emit_stats_part(ep + 1, 3, st_next)

                # the Sqrt (and its two ACT table reloads) lands in the
                # proj/transpose window where the ACT engine runs no Exp
                if el == 1 and ep + 1 < npairs:
                    xn_next = emit_norm(st_next["x_ts"], st_next["mv"])

                # ---- proj: y = O Wp ----  (next pair's transposes are
                # emitted between the two proj chains to fill PE gaps)
                for nci in range(2):
                    y_ps = ps_oty.tile([128, 512], f32, tag="oty", name="y_ps")
                    for slot in range(H):
                        nc.tensor.matmul(y_ps,
                                         lhsT=ot_sb[:, slot, nci * 128:(nci + 1) * 128],
                                         rhs=wp_sb[:, HEAD_ORDER[slot], :],
                                         start=(slot == 0), stop=(slot == H - 1))
                    yb_sb = sb_y.tile([128, DIM], f32, tag="yb")
                    if use_bp:
                        nc.vector.tensor_tensor(out=yb_sb, in0=y_ps,
                                                in1=bp_sb[:, 0, :], op=Alu.add)
                    else:
                        # DVE, not ACT: these land right before the next
                        # elem's Exp burst on the ACT queue
                        nc.vector.tensor_copy(out=yb_sb, in_=y_ps)
                    nc.sync.dma_start(out=y_d[e, nci * 128:(nci + 1) * 128, :],
                                      in_=yb_sb)
                    if el == 1 and ep + 1 < npairs:
                        if nci == 0:
                            zT_next = alloc_zT()
                        emit_transposes(zT_next, xn_next, nci)

            xn_cur = xn_next
            zT_cur = zT_next

    nc.compile()
    return nc


def _prepare(x, gamma, beta, Wqkv, bqkv, Wproj, bproj, biases, bias_idxs):
    x = np.ascontiguousarray(np.asarray(x, dtype=np.float32))
    gamma = np.asarray(gamma, dtype=np.float32)
    beta = np.asarray(beta, dtype=np.float32)
    Wqkv = np.asarray(Wqkv, dtype=np.float32)
    bqkv = np.asarray(bqkv, dtype=np.float32)
    Wproj = np.asarray(Wproj, dtype=np.float32)
    bproj = np.asarray(bproj, dtype=np.float32)
    biases = np.asarray(biases, dtype=np.float32)
    bias_idxs = np.asarray(bias_idxs)

    s = np.float32(KD ** -0.5)
    Wg = Wqkv * gamma[:, None]
    bfull = beta @ Wqkv + bqkv
    Wr = Wg.reshape(DIM, H, 64 + D)
    br = bfull.reshape(H, 64 + D)
    # feature layout (see kernel comment): head h -> strip h%3; q in chunk
    # h//3, k in chunk 4 + h//3.
    wqk = np.zeros((DIM, 8, 128), dtype=np.float32)
    bqk = np.zeros((8, 128), dtype=np.float32)
    for h in range(H):
        qc, base = h // 3, (h % 3) * KD
        wqk[:, qc, base:base + KD] = Wr[:, h, 0:KD] * s
        wqk[:, 4 + qc, base:base + KD] = Wr[:, h, KD:2 * KD]
        bqk[qc, base:base + KD] = br[h, 0:KD] * s
        bqk[4 + qc, base:base + KD] = br[h, KD:2 * KD]
    wqk = np.ascontiguousarray(wqk.reshape(DIM, 8 * 128)).astype(np.float16)
    wv = np.ascontiguousarray(
        Wr[:, :, 2 * KD:].reshape(DIM, DH)).astype(np.float16)
    bv = br[:, 2 * KD:].reshape(DH)
    bp = bproj + bv @ Wproj
    expb = np.exp(biases[:, bias_idxs])  # [H, N, N]
    # head dim reordered to the kernel's strip-pure processing order
    expb_t = np.ascontiguousarray(
        expb[HEAD_ORDER].reshape(H, 2, 128, N).transpose(2, 1, 0, 3)
    ).astype(np.float16)

    use_bqk = bool(np.abs(bqk).max() > 0)
    use_bp = bool(np.abs(bp).max() > 0)
    bqk_t = np.ascontiguousarray(bqk.T)  # [128, 8]

    common = {"wqk": wqk, "wv": wv,
              "wp": np.ascontiguousarray(Wproj).astype(np.float16),
              "expb": expb_t, "ones": np.ones((128, 1), dtype=np.float16)}
    if use_bqk:
        common["bqk"] = bqk_t
    if use_bp:
        common["bp"] = np.ascontiguousarray(bp)
    in_maps = []
    for c in range(NCORES):
        m = dict(common)
        m["x"] = np.ascontiguousarray(x[c * BPC:(c + 1) * BPC])
        in_maps.append(m)
    return in_maps, use_bqk, use_bp


def run(inputs, trace=False, **run_kwargs):
    from concourse.bass_utils import run_bass_kernel_spmd

    in_maps, use_bqk, use_bp = _prepare(**inputs)
    key = (BPC, use_bqk, use_bp)
    if key not in _CACHE:
        _CACHE[key] = _build(*key)
    nc = _CACHE[key]
    res = run_bass_kernel_spmd(nc, in_maps, core_ids=list(range(NCORES)),
                               trace=trace, **run_kwargs)
    y = np.concatenate([res.results[c]["y"] for c in range(NCORES)], axis=0)
    return y, res


def kernel(**inputs):
    y, _ = run(inputs)
    return y
